# revision 1
# baseline (speedup 1.0000x reference)
"""Multi-head attention (B=4, S=2048, D=1024, H=16, causal + RoPE) on 8 trn2 cores.

Sharding: head-parallel. Core c owns heads {2c, 2c+1}:
  - computes Q/K/V projections for its 2 heads over all B*S positions,
  - RoPE + causal softmax attention,
  - row-parallel partial out-projection y_c^T = (attn_out_c @ w_out[128c:128c+128]).T
Host sums the 8 partials and transposes back (the unshard for row-parallel TP).

On-chip dataflow is fully transposed:
  xT [D, B*S] -> QT/KT [128(2h*64), B*S] (RoPE'd) -> scores^T [j,q] -> exp ->
  AV with ones-column (unnormalized out^T + softmax denominator in one matmul) ->
  normalize -> out-proj -> yT [D, B*S] DMA'd straight from PSUM.
"""

import numpy as np

# ---- fixed problem config ----
B, S, D = 4, 2048, 1024
H, HD = 16, 64
NCORES = 8
HPC = H // NCORES          # heads per core = 2
ROPE_BASE = 10000.0

QBLK = 512                 # query block (free dim of scores/AV matmuls)
JCH = 128                  # key chunk (partition dim of scores^T)
NEG = -30000.0             # additive mask value (exp underflows to 0)


# --------------------------------------------------------------------------
# host-side helpers
# --------------------------------------------------------------------------

def _rope_tables_T(s, hd, hpc):
    """cos/sin tables, transposed [hpc*hd, s], sin with rotate-half sign folded in.

    Matches reference._rope_cache computed in float32.
    """
    inv_freq = 1.0 / (ROPE_BASE ** (np.arange(0, hd, 2, dtype=np.float32) / np.float32(hd)))
    t = np.arange(s, dtype=np.float32)
    freqs = np.outer(t, inv_freq).astype(np.float32)          # [s, hd/2]
    emb = np.concatenate([freqs, freqs], axis=-1)             # [s, hd]
    cos = np.cos(emb).T.astype(np.float32)                    # [hd, s]
    sin = np.sin(emb).T.astype(np.float32)
    # q_rot = q*cos + rothalf(q)*sin ; rothalf(q)[d] = -q[d+32] (d<32), +q[d-32]
    # we build tmp[d] = q[partner(d)] with plain copies, so fold the sign into sin
    sin_signed = np.concatenate([-sin[: hd // 2], sin[hd // 2 :]], axis=0)
    return (
        np.tile(cos, (hpc, 1)).copy(),        # [hpc*hd, s]
        np.tile(sin_signed, (hpc, 1)).copy(),
    )


def _master_mask(qblk):
    """master[j, u] = 1.0 iff u >= j + 384, shape [128, 384 + qblk].

    For a diagonal chunk with offset o = jc*JCH - q0 in {0,128,256,384}, the
    causal keep-mask for tile [j in chunk, q in block] is master[:, 384-o : 384-o+qblk].
    """
    j = np.arange(JCH)[:, None]
    u = np.arange(384 + qblk)[None, :]
    return (u >= j + 384).astype(np.float32)


def _key_bias(attention_mask, s):
    """[128, B*(s//128)] additive bias per key position: 0 if valid, NEG if padded."""
    b = attention_mask.shape[0]
    kb = np.where(np.asarray(attention_mask) == 0, np.float32(NEG), np.float32(0.0))
    kb = kb.reshape(b * (s // JCH), JCH).T.astype(np.float32)   # [128, b*s/128]
    return np.ascontiguousarray(kb)


# --------------------------------------------------------------------------
# device program
# --------------------------------------------------------------------------

def emit(tc, outs, ins, *, b, s, d, mm="f32"):
    """Emit the per-core program into TileContext tc.

    ins/outs: dicts of DRAM APs:
      ins:  xT [d, b*s], wq/wk/wv [d, 128], wo [128, d],
            cosT/sinTs [128, s], master [128, 384+QBLK], keybias [128, b*s/128]
      outs: yT [d, b*s]
    mm: matmul operand mode: "f32" (exact, 4 cyc/row) or "f32r" (fast, ~tf32).
    """
    import concourse.bass as bass
    import concourse.mybir as mybir
    from concourse.masks import make_identity

    nc = tc.nc
    f32 = mybir.dt.float32
    f32r = mybir.dt.float32r
    AF = mybir.ActivationFunctionType

    def mmcast(ap):
        return ap.bitcast(f32r) if mm == "f32r" else ap

    bs = b * s
    kchunks = d // 128          # contraction chunks for projections
    ngroups = bs // QBLK        # 512-wide position groups
    nt = bs // JCH              # 128-wide position chunks (vones index)
    nqb = s // QBLK             # query blocks per sequence
    njd = QBLK // JCH           # j-chunks per query block (diagonal count) = 4
    nnch = d // 128             # out-proj n chunks

    xT, wq, wk, wv, wo = ins["xT"], ins["wq"], ins["wk"], ins["wv"], ins["wo"]
    cosT, sinTs, master, keybias = ins["cosT"], ins["sinTs"], ins["master"], ins["keybias"]
    yT = outs["yT"]

    import contextlib
    ctx = contextlib.ExitStack()
    with ctx:
        singles = ctx.enter_context(tc.tile_pool(name="singles", bufs=1))
        xpool = ctx.enter_context(tc.tile_pool(name="xtiles", bufs=2))
        ps_pool = ctx.enter_context(tc.tile_pool(name="ps", bufs=2, space="PSUM"))
        ptr_pool = ctx.enter_context(tc.tile_pool(name="ptr", bufs=2, space="PSUM"))
        pav_pool = ctx.enter_context(tc.tile_pool(name="pav", bufs=2, space="PSUM"))
        py_pool = ctx.enter_context(tc.tile_pool(name="py", bufs=2, space="PSUM"))
        tmp_pool = ctx.enter_context(tc.tile_pool(name="tmp", bufs=2))
        ypool = ctx.enter_context(tc.tile_pool(name="yev", bufs=2))
        ex_pool = ctx.enter_context(tc.tile_pool(name="ex", bufs=3))
        outh_pool = ctx.enter_context(tc.tile_pool(name="outh", bufs=2))
        bc_pool = ctx.enter_context(tc.tile_pool(name="bc", bufs=2))

        # ---- persistent SBUF state ----
        wq_sb = singles.tile([128, kchunks, 128], f32)
        wk_sb = singles.tile([128, kchunks, 128], f32)
        wv_sb = singles.tile([128, kchunks, 128], f32)
        wo_sb = singles.tile([128, nnch, 128], f32)
        cos_sb = singles.tile([128, s], f32)
        sin_sb = singles.tile([128, s], f32)
        mst_sb = singles.tile([128, 384 + QBLK], f32)
        kb_sb = singles.tile([128, nt], f32)
        ident = singles.tile([128, 128], f32)
        qT_sb = singles.tile([128, bs], f32)
        kT_sb = singles.tile([128, bs], f32)
        # vones: stationary for AV matmul, per head: [V | ones]
        # out rows 0..63 = unnormalized out^T, row 64 = softmax denominator
        v0_sb = singles.tile([128, nt, 65], f32)
        v1_sb = singles.tile([128, nt, 65], f32)

        for kc in range(kchunks):
            nc.sync.dma_start(out=wq_sb[:, kc, :], in_=wq[kc * 128 : (kc + 1) * 128, :])
            nc.sync.dma_start(out=wk_sb[:, kc, :], in_=wk[kc * 128 : (kc + 1) * 128, :])
            nc.sync.dma_start(out=wv_sb[:, kc, :], in_=wv[kc * 128 : (kc + 1) * 128, :])
        nc.sync.dma_start(out=wo_sb[:, :, :], in_=wo.rearrange("c (n m) -> c n m", m=128))
        nc.sync.dma_start(out=cos_sb[:, :], in_=cosT[:, :])
        nc.sync.dma_start(out=sin_sb[:, :], in_=sinTs[:, :])
        nc.sync.dma_start(out=mst_sb[:, :], in_=master[:, :])
        nc.sync.dma_start(out=kb_sb[:, :], in_=keybias[:, :])
        make_identity(nc, ident[:, :])
        nc.vector.memset(v0_sb[:, :, 64:65], 1.0)
        nc.vector.memset(v1_sb[:, :, 64:65], 1.0)

        # ---- phase A: QKV projections (transposed layout) + RoPE + V transpose ----
        for g in range(ngroups):
            g0 = g * QBLK                 # global position offset
            ps0 = g0 % s                  # position within sequence (for rope tables)
            xt = xpool.tile([128, kchunks, QBLK], f32, tag="xt")
            for kc in range(kchunks):
                nc.sync.dma_start(
                    out=xt[:, kc, :], in_=xT[kc * 128 : (kc + 1) * 128, g0 : g0 + QBLK]
                )

            for which, w_sb, dst in (("q", wq_sb, qT_sb), ("k", wk_sb, kT_sb)):
                pp = ps_pool.tile([128, QBLK], f32, tag="ps")
                for kc in range(kchunks):
                    nc.tensor.matmul(
                        pp[:, :],
                        mmcast(w_sb[:, kc, :]),
                        mmcast(xt[:, kc, :]),
                        start=(kc == 0),
                        stop=(kc == kchunks - 1),
                    )
                # RoPE: dst = raw*cos + swap_halves(raw)*sin_signed
                raw = tmp_pool.tile([128, QBLK], f32, tag="raw")
                nc.scalar.copy(out=raw[:, :], in_=pp[:, :])
                tmp = tmp_pool.tile([128, QBLK], f32, tag="rope_tmp")
                for hb in (0, 64):
                    nc.sync.dma_start(out=tmp[hb : hb + 32, :], in_=raw[hb + 32 : hb + 64, :])
                    nc.sync.dma_start(out=tmp[hb + 32 : hb + 64, :], in_=raw[hb : hb + 32, :])
                nc.vector.tensor_mul(tmp[:, :], tmp[:, :], sin_sb[:, ps0 : ps0 + QBLK])
                nc.vector.tensor_mul(dst[:, g0 : g0 + QBLK], raw[:, :], cos_sb[:, ps0 : ps0 + QBLK])
                nc.vector.tensor_add(dst[:, g0 : g0 + QBLK], dst[:, g0 : g0 + QBLK], tmp[:, :])

            # V: project (transposed), evacuate, PE-transpose into vones natural layout
            pv = ps_pool.tile([128, QBLK], f32, tag="ps")
            for kc in range(kchunks):
                nc.tensor.matmul(
                    pv[:, :],
                    mmcast(wv_sb[:, kc, :]),
                    mmcast(xt[:, kc, :]),
                    start=(kc == 0),
                    stop=(kc == kchunks - 1),
                )
            vt = tmp_pool.tile([128, QBLK], f32, tag="vt")
            nc.scalar.copy(out=vt[:, :], in_=pv[:, :])
            for c in range(QBLK // JCH):
                t = g * (QBLK // JCH) + c
                ptr0 = ptr_pool.tile([128, 64], f32, tag="ptr")
                nc.tensor.transpose(
                    ptr0[:, :], vt[0:64, c * JCH : (c + 1) * JCH], ident[0:64, 0:64]
                )
                nc.vector.tensor_copy(v0_sb[:, t, 0:64], ptr0[:, :])
                ptr1 = ptr_pool.tile([128, 64], f32, tag="ptr")
                nc.tensor.transpose(
                    ptr1[:, :], vt[64:128, c * JCH : (c + 1) * JCH], ident[64:128, 64:128]
                )
                nc.vector.tensor_copy(v1_sb[:, t, 0:64], ptr1[:, :])

        # ---- phases C+D: attention + out-projection, per (batch, query block) ----
        for bi in range(b):
            for qb in range(nqb):
                q0 = bi * s + qb * QBLK      # global column offset
                outh = outh_pool.tile([128, QBLK], f32, tag="outh")
                for h in (0, 1):
                    hb = h * 64
                    pav = pav_pool.tile([128, QBLK], f32, tag="pav")
                    av_out = pav[0:65, :]
                    nj = njd * (qb + 1)
                    for jc in range(nj):
                        jg = bi * s + jc * JCH
                        pss = ps_pool.tile([128, QBLK], f32, tag="ps")
                        nc.tensor.matmul(
                            pss[:, :],
                            mmcast(kT_sb[hb : hb + 64, jg : jg + JCH]),
                            mmcast(qT_sb[hb : hb + 64, q0 : q0 + QBLK]),
                            start=True,
                            stop=True,
                        )
                        ex = ex_pool.tile([128, QBLK], f32, tag="ex")
                        kbi = bi * (s // JCH) + jc
                        nc.scalar.activation(
                            out=ex[:, :],
                            in_=pss[:, :],
                            func=AF.Exp,
                            bias=kb_sb[:, kbi : kbi + 1],
                            scale=float(1.0 / np.sqrt(HD)),
                        )
                        if jc >= njd * qb:  # diagonal chunk: causal mask
                            off = 384 - (jc - njd * qb) * JCH
                            nc.vector.tensor_mul(
                                ex[:, :], ex[:, :], mst_sb[:, off : off + QBLK]
                            )
                        vsl = v0_sb[:, jc + bi * (s // JCH), :] if h == 0 else v1_sb[:, jc + bi * (s // JCH), :]
                        nc.tensor.matmul(
                            av_out,
                            mmcast(vsl),
                            mmcast(ex[:, :]),
                            start=(jc == 0),
                            stop=(jc == nj - 1),
                        )
                    # normalize: divide data rows by the denominator row, broadcast over partitions
                    bct = bc_pool.tile([128, QBLK], f32, tag="bc")
                    nc.vector.reciprocal(bct[64:65, :], pav[64:65, :])
                    row = bct[64:65, :]
                    bc3 = bass.AP(tensor=row.tensor, offset=row.offset,
                                  ap=[list(row.ap[0])] + [[0, 64]] + [list(row.ap[1])])
                    nc.sync.dma_start(out=bct[0:64, :], in_=bc3)
                    if h == 0:
                        nc.vector.tensor_mul(outh[0:64, :], pav[0:64, :], bct[0:64, :])
                    else:
                        oh1 = bc_pool.tile([64, QBLK], f32, tag="oh1")
                        nc.vector.tensor_mul(oh1[:, :], pav[0:64, :], bct[0:64, :])
                        nc.sync.dma_start(out=outh[64:128, :], in_=oh1[:, :])
                # out-projection for this (batch, query block)
                for n in range(nnch):
                    py = py_pool.tile([128, QBLK], f32, tag="py")
                    nc.tensor.matmul(
                        py[:, :], mmcast(wo_sb[:, n, :]), mmcast(outh[:, :]), start=True, stop=True
                    )
                    ysb = ypool.tile([128, QBLK], f32, tag="yevac")
                    nc.vector.tensor_copy(ysb[:, :], py[:, :])
                    nc.sync.dma_start(out=yT[n * 128 : (n + 1) * 128, q0 : q0 + QBLK], in_=ysb[:, :])


# --------------------------------------------------------------------------
# host entry point
# --------------------------------------------------------------------------

def _shard_inputs(x, attention_mask, w_qkv, w_out, b, s, d):
    """Build the per-core input maps (host-side shard/prep)."""
    xT = np.ascontiguousarray(np.asarray(x, dtype=np.float32).reshape(b * s, d).T)
    w_qkv = np.asarray(w_qkv, dtype=np.float32)
    w_out = np.asarray(w_out, dtype=np.float32)
    cosT, sinTs = _rope_tables_T(s, HD, HPC)
    master = _master_mask(QBLK)
    keybias = _key_bias(attention_mask, s)
    cw = HPC * HD  # 128 columns per core
    in_maps = []
    for c in range(NCORES):
        sl = slice(c * cw, (c + 1) * cw)
        in_maps.append(
            {
                "xT": xT,
                "wq": np.ascontiguousarray(w_qkv[:, 0 * d :][:, sl]),
                "wk": np.ascontiguousarray(w_qkv[:, 1 * d :][:, sl]),
                "wv": np.ascontiguousarray(w_qkv[:, 2 * d :][:, sl]),
                "wo": np.ascontiguousarray(w_out[sl, :]),
                "cosT": cosT,
                "sinTs": sinTs,
                "master": master,
                "keybias": keybias,
            }
        )
    return in_maps


_PROG_CACHE = {}


def _build_program(b, s, d, mm):
    key = (b, s, d, mm)
    if key in _PROG_CACHE:
        return _PROG_CACHE[key]
    import concourse.mybir as mybir
    from concourse import bacc
    from concourse.tile import TileContext

    f32 = mybir.dt.float32
    nc = bacc.Bacc("TRN2", target_bir_lowering=False, debug=False)
    bs = b * s
    ins = {
        "xT": nc.dram_tensor("xT", [d, bs], f32, kind="ExternalInput").ap(),
        "wq": nc.dram_tensor("wq", [d, 128], f32, kind="ExternalInput").ap(),
        "wk": nc.dram_tensor("wk", [d, 128], f32, kind="ExternalInput").ap(),
        "wv": nc.dram_tensor("wv", [d, 128], f32, kind="ExternalInput").ap(),
        "wo": nc.dram_tensor("wo", [128, d], f32, kind="ExternalInput").ap(),
        "cosT": nc.dram_tensor("cosT", [128, s], f32, kind="ExternalInput").ap(),
        "sinTs": nc.dram_tensor("sinTs", [128, s], f32, kind="ExternalInput").ap(),
        "master": nc.dram_tensor("master", [128, 384 + QBLK], f32, kind="ExternalInput").ap(),
        "keybias": nc.dram_tensor("keybias", [128, bs // JCH], f32, kind="ExternalInput").ap(),
    }
    outs = {"yT": nc.dram_tensor("yT", [d, bs], f32, kind="ExternalOutput").ap()}
    with TileContext(nc) as tc:
        emit(tc, outs, ins, b=b, s=s, d=d, mm=mm)
    nc.compile()
    _PROG_CACHE[key] = nc
    return nc


def kernel(x, attention_mask, w_qkv, w_out, *, mm="f32", trace=False):
    from concourse import bass_utils

    b, s, d = x.shape
    nc = _build_program(b, s, d, mm)
    in_maps = _shard_inputs(x, attention_mask, w_qkv, w_out, b, s, d)
    res = bass_utils.run_bass_kernel_spmd(
        nc, in_maps, core_ids=list(range(NCORES)), trace=trace
    )
    acc = res.results[0]["yT"].astype(np.float32)
    for c in range(1, NCORES):
        acc = acc + res.results[c]["yT"]
    out = np.ascontiguousarray(acc.T).reshape(b, s, d).astype(np.float32)
    if trace:
        return out, res
    return out



# revision 18
# speedup vs baseline: 2.0238x; 2.0238x over previous
"""Multi-head attention (B=4, S=2048, D=1024, H=16, causal + RoPE) on 8 trn2 cores.

Sharding: head-parallel. Core c owns heads {2c, 2c+1}:
  - Q/K/V projections for its 2 heads over all B*S positions,
  - RoPE + causal softmax attention,
  - row-parallel partial out-projection; host sums the 8 bf16 partials.

v2 design notes (instruction-count-driven; each matmul ~230ns fixed, each
dma_start ~625ns of serialized issue):
  - block pipeline: per (batch, 512-query-block): project q/k/v -> RoPE ->
    attention -> out-proj; block k's projection is emitted before block k-1's
    attention so PE never waits on RoPE.
  - RoPE rotate-half partners are placed on adjacent partitions by permuting
    the q/k weight columns host-side, so the half-swap is a single DVE
    stream_shuffle (mask [1,0,3,2,...]) instead of DMAs.
  - V is transposed into key-major vones layout with dma_start_transpose.
  - j-chunks processed in pairs: 2 score matmuls -> one [128,1024] exp -> 2 AV
    matmuls; diagonal chunks use column subranges; causal mask multiplies only
    the triangle strips.
  - softmax denominator via an extra vones column that carries the key-padding
    mask (1.0/0.0); the reciprocal row is broadcast across partitions with a
    K=1 matmul into PSUM instead of a broadcast DMA.
  - out partials written as bf16 (host accumulates in f32).
"""

import numpy as np

# ---- fixed problem config ----
B, S, D = 4, 2048, 1024
H, HD = 16, 64
NCORES = 8
HPC = H // NCORES          # heads per core = 2
ROPE_BASE = 10000.0

QBLK = 512                 # query block (free dim of scores/AV matmuls)
JCH = 128                  # key chunk (partition dim of scores^T)


# --------------------------------------------------------------------------
# host-side helpers
# --------------------------------------------------------------------------

def _perm64():
    """Rotate-half pairing permutation: head-dim i -> 2i, i+32 -> 2i+1."""
    p = np.empty(64, dtype=np.int64)
    p[np.arange(32) * 2] = np.arange(32)          # even slots <- dims 0..31
    p[np.arange(32) * 2 + 1] = np.arange(32, 64)  # odd slots  <- dims 32..63
    return p                                       # p[slot] = orig dim


def _rope_tables_T(s, hd, hpc):
    """cos/sin tables [hpc*hd, s] in permuted row order, sin sign-folded.

    After permutation, partition 2i holds dim i and partition 2i+1 holds dim
    i+32 (per head block of 64). rot-half swap = swap adjacent partitions;
    sign: even slots get -sin, odd slots +sin. cos/sin rows use freq of
    dim mod 32 (emb = concat(freqs, freqs)).
    """
    inv_freq = 1.0 / (ROPE_BASE ** (np.arange(0, hd, 2, dtype=np.float32) / np.float32(hd)))
    t = np.arange(s, dtype=np.float32)
    freqs = np.outer(t, inv_freq).astype(np.float32)          # [s, hd/2]
    emb = np.concatenate([freqs, freqs], axis=-1)             # [s, hd]
    cos = np.cos(emb).T.astype(np.float32)                    # [hd, s]
    sin = np.sin(emb).T.astype(np.float32)
    perm = _perm64()
    cosp = cos[perm]                                          # [hd, s] permuted
    sinp = sin[perm]
    sign = np.where(np.arange(hd) % 2 == 0, np.float32(-1.0), np.float32(1.0))
    sinp = sinp * sign[:, None]
    return (
        np.ascontiguousarray(np.tile(cosp, (hpc, 1))),        # [hpc*hd, s]
        np.ascontiguousarray(np.tile(sinp, (hpc, 1))),
    )


def _master_mask(qblk):
    """master[j, u] = 1.0 iff u >= j + 384, shape [128, 384 + qblk].

    Triangle strip for any diagonal chunk is master[:, 384:512]; the c=3
    256-wide strip (zero block + triangle) is master[:, 256:512].
    """
    j = np.arange(JCH)[:, None]
    u = np.arange(384 + qblk)[None, :]
    return (u >= j + 384).astype(np.float32)


# --------------------------------------------------------------------------
# device program
# --------------------------------------------------------------------------

def emit(tc, outs, ins, *, b, s, d, mm="f32r", has_padding=False):
    import concourse.bass as bass
    import concourse.mybir as mybir

    nc = tc.nc
    f32 = mybir.dt.float32
    f32r = mybir.dt.float32r
    bf16 = mybir.dt.bfloat16
    AF = mybir.ActivationFunctionType
    mf = f32r if mm == "f32r" else f32

    bs = b * s
    kchunks = d // 128          # 8 contraction chunks for projections
    nqb = s // QBLK             # 4 query blocks per sequence
    njd = QBLK // JCH           # 4 j-chunks per query block
    ntseq = s // JCH            # 16 key chunks per sequence
    nnch = d // 128             # 8 out-proj n chunks
    scale = float(1.0 / np.sqrt(HD))

    xT, wqkv, wo = ins["xT"], ins["wqkv"], ins["wo"]
    cosT, sinTs, master, kmT = (
        ins["cosT"], ins["sinTs"], ins["master"], ins["kmT"],
    )
    yT = outs["yT"]

    # swap-adjacent-partitions shuffle mask
    SWAP_MASK = [i ^ 1 for i in range(32)]

    def sub2(ap2d, start, stride, n, w):
        """[128, n, w] AP over free columns {start + i*stride + j}."""
        sl = ap2d[:, start : start + stride * (n - 1) + w]
        return bass.AP(
            tensor=sl.tensor, offset=sl.offset,
            ap=[list(sl.ap[0])] + [[stride, n]] + [[1, w]],
        )

    import contextlib
    ctx = contextlib.ExitStack()
    with ctx:
        singles = ctx.enter_context(tc.tile_pool(name="singles", bufs=1))
        xpool = ctx.enter_context(tc.tile_pool(name="xtiles", bufs=2))
        ps_pool = ctx.enter_context(tc.tile_pool(name="ps", bufs=2, space="PSUM"))
        pav_pool = ctx.enter_context(tc.tile_pool(name="pav", bufs=2, space="PSUM"))
        pout_pool = ctx.enter_context(tc.tile_pool(name="pout", bufs=2, space="PSUM"))
        tmp_pool = ctx.enter_context(tc.tile_pool(name="tmp", bufs=2))
        qt_pool = ctx.enter_context(tc.tile_pool(name="qt", bufs=2))
        vt_pool = ctx.enter_context(tc.tile_pool(name="vt", bufs=2))
        ex_pool = ctx.enter_context(tc.tile_pool(name="ex", bufs=3))
        bct_pool = ctx.enter_context(tc.tile_pool(name="bct", bufs=2))
        bcs_pool = ctx.enter_context(tc.tile_pool(name="bcs", bufs=2))
        outh_pool = ctx.enter_context(tc.tile_pool(name="outh", bufs=2))
        ysb_pool = ctx.enter_context(tc.tile_pool(name="ysb", bufs=2))

        # ---- persistent SBUF state ----
        wqkv_sb = singles.tile([128, kchunks, 3, 128], mf)
        wo_sb = singles.tile([128, nnch, 128], mf)
        cos_sb = singles.tile([128, s], f32)
        sin_sb = singles.tile([128, s], f32)
        mst_sb = singles.tile([128, 384 + QBLK], f32)
        kT_sb = singles.tile([128, s], mf)
        # vones per sequence: h0 cols [V(0:64) | km(64)], h1 cols [km(0) | V(1:65)]
        v0_sb = singles.tile([128, ntseq, 65], mf)
        v1_sb = singles.tile([128, ntseq, 65], mf)

        nc.sync.dma_start(
            out=wqkv_sb[:, :, :, :],
            in_=wqkv.rearrange("(kc p) t n -> p kc t n", p=128),
        )
        nc.sync.dma_start(out=wo_sb[:, :, :], in_=wo.rearrange("c (n m) -> c n m", m=128))
        nc.sync.dma_start(out=cos_sb[:, :], in_=cosT[:, :])
        nc.sync.dma_start(out=sin_sb[:, :], in_=sinTs[:, :])
        nc.sync.dma_start(out=mst_sb[:, :], in_=master[:, :])

        xTr = xT.rearrange("(kc p) q -> p kc q", p=128)
        yTr = yT.rearrange("(n p) q -> p n q", p=128)

        # one iteration = emit proj+rope for block k, then attention for k-1
        nblk = b * nqb

        def emit_proj(k):
            bi, qb = divmod(k, nqb)
            g0 = bi * s + qb * QBLK
            ps0 = qb * QBLK
            t0 = qb * njd
            if qb == 0:
                # (re)load km column of vones for this sequence
                nc.sync.dma_start(out=v0_sb[:, :, 64:65], in_=kmT[:, bi, :, :])
                nc.sync.dma_start(out=v1_sb[:, :, 64:65], in_=kmT[:, bi, :, :])
            xt = xpool.tile([128, kchunks, QBLK], mf, tag="xt")
            nc.sync.dma_start(out=xt[:, :, :], in_=xTr[:, :, g0 : g0 + QBLK])
            psv = ps_pool.tile([128, 2 * QBLK], f32, tag="ps")
            psq = ps_pool.tile([128, 2 * QBLK], f32, tag="ps")
            psk = ps_pool.tile([128, 2 * QBLK], f32, tag="ps")
            for dst, ti in ((psv, 2), (psq, 0), (psk, 1)):
                for kc in range(kchunks):
                    nc.tensor.matmul(
                        dst[:, 0:QBLK],
                        wqkv_sb[:, kc, ti, :],
                        xt[:, kc, :],
                        start=(kc == 0),
                        stop=(kc == kchunks - 1),
                    )
            # V evacuation (cast bf16 for the 2-byte XBAR transpose) + upcast copies
            vt = vt_pool.tile([128, QBLK], bf16, tag="vt")
            nc.scalar.copy(out=vt[:, :], in_=psv[:, 0:QBLK])
            vtr0 = vt_pool.tile([128, njd, 64], bf16, tag="vtr0")
            vtr1 = vt_pool.tile([128, njd, 64], bf16, tag="vtr1")
            nc.scalar.dma_start_transpose(out=vtr0[:, :, :], in_=vt[0:64, :])
            nc.scalar.dma_start_transpose(out=vtr1[:, :, :], in_=vt[64:128, :])
            nc.vector.tensor_copy(v0_sb[:, t0 : t0 + njd, 0:64], vtr0[:, :, :])
            nc.vector.tensor_copy(v1_sb[:, t0 : t0 + njd, 0:64], vtr1[:, :, :])
            if has_padding:
                for c in range(njd):
                    t = t0 + c
                    km0 = bass.AP(
                        tensor=v0_sb.tensor, offset=v0_sb[:, t, 64:65].offset,
                        ap=[list(v0_sb[:, t, 64:65].ap[0])] + [[0, 65]],
                    )
                    nc.vector.tensor_mul(v0_sb[:, t, 0:65], v0_sb[:, t, 0:65], km0)
                    km1 = bass.AP(
                        tensor=v1_sb.tensor, offset=v1_sb[:, t, 64:65].offset,
                        ap=[list(v1_sb[:, t, 64:65].ap[0])] + [[0, 65]],
                    )
                    nc.vector.tensor_mul(v1_sb[:, t, 0:65], v1_sb[:, t, 0:65], km1)
            # RoPE: partners are adjacent partitions -> stream_shuffle swap
            tmp = tmp_pool.tile([128, 2 * QBLK], f32, tag="tmp")
            nc.vector.stream_shuffle(tmp[:, 0:QBLK], psq[:, 0:QBLK], SWAP_MASK)
            nc.vector.stream_shuffle(tmp[:, QBLK : 2 * QBLK], psk[:, 0:QBLK], SWAP_MASK)
            nc.vector.tensor_mul(tmp[:, 0:QBLK], tmp[:, 0:QBLK], sin_sb[:, ps0 : ps0 + QBLK])
            nc.vector.tensor_mul(
                tmp[:, QBLK : 2 * QBLK], tmp[:, QBLK : 2 * QBLK], sin_sb[:, ps0 : ps0 + QBLK]
            )
            qt = qt_pool.tile([128, QBLK], mf, tag="qt")
            nc.vector.tensor_mul(qt[:, :], psq[:, 0:QBLK], cos_sb[:, ps0 : ps0 + QBLK])
            nc.vector.tensor_add(qt[:, :], qt[:, :], tmp[:, 0:QBLK])
            ksl = kT_sb[:, ps0 : ps0 + QBLK]
            nc.vector.tensor_mul(ksl, psk[:, 0:QBLK], cos_sb[:, ps0 : ps0 + QBLK])
            nc.vector.tensor_add(ksl, ksl, tmp[:, QBLK : 2 * QBLK])
            return qt

        def emit_attn(k, qt):
            bi, qb = divmod(k, nqb)
            g0 = bi * s + qb * QBLK
            nj = njd * (qb + 1)
            jdiag0 = njd * qb           # first diagonal chunk index
            # column subrange starts per diagonal index c (AV/scores width >= 256)
            DCOL = (0, 128, 256, 256)
            pavs = []
            for h in (0, 1):
                hb = h * 64
                pav = pav_pool.tile([128, QBLK], f32, tag="pav")
                pavs.append(pav)
                r0 = 0
                vsb = v0_sb if h == 0 else v1_sb
                for pr in range(nj // 2):
                    jc0 = 2 * pr
                    E = ps_pool.tile([128, 2 * QBLK], f32, tag="ps")
                    ex = ex_pool.tile([128, 2 * QBLK], mf, tag="ex")
                    cols = []
                    for i in (0, 1):
                        jc = jc0 + i
                        c = jc - jdiag0
                        col0 = DCOL[c] if c >= 0 else 0
                        cols.append(col0)
                        nc.tensor.matmul(
                            E[:, i * QBLK + col0 : (i + 1) * QBLK],
                            kT_sb[hb : hb + 64, jc * JCH : (jc + 1) * JCH],
                            qt[hb : hb + 64, col0:QBLK],
                            start=True,
                            stop=True,
                        )
                    # exp (scale folded); subrange AP when both chunks start at 256
                    if cols[0] == 256 and cols[1] == 256:
                        nc.scalar.activation(
                            out=sub2(ex, 256, QBLK, 2, 256),
                            in_=sub2(E, 256, QBLK, 2, 256),
                            func=AF.Exp,
                            scale=scale,
                        )
                    else:
                        nc.scalar.activation(
                            out=ex[:, :], in_=E[:, :], func=AF.Exp, scale=scale
                        )
                    # causal masks on diagonal chunks
                    for i in (0, 1):
                        jc = jc0 + i
                        c = jc - jdiag0
                        if c >= 0:
                            if c == 3:
                                nc.vector.tensor_mul(
                                    ex[:, i * QBLK + 256 : (i + 1) * QBLK],
                                    ex[:, i * QBLK + 256 : (i + 1) * QBLK],
                                    mst_sb[:, 256:512],
                                )
                            else:
                                tc0 = i * QBLK + c * JCH
                                nc.vector.tensor_mul(
                                    ex[:, tc0 : tc0 + JCH],
                                    ex[:, tc0 : tc0 + JCH],
                                    mst_sb[:, 384:512],
                                )
                    for i in (0, 1):
                        jc = jc0 + i
                        col0 = cols[i]
                        nc.tensor.matmul(
                            pav[r0 : r0 + 65, col0:QBLK],
                            vsb[:, jc, 0:65],
                            ex[:, i * QBLK + col0 : (i + 1) * QBLK],
                            start=(jc == 0),
                            stop=(jc == nj - 1),
                            skip_group_check=True,
                        )
            # normalize: reciprocal of denominator row, 0-stride broadcast DMA, mul
            outh = outh_pool.tile([128, QBLK], mf, tag="outh")
            for h in (0, 1):
                bct = bct_pool.tile([128, QBLK], f32, tag=f"bct{h}")
                nc.vector.reciprocal(bct[64:65, :], pavs[h][64:65, :])
                row = bct[64:65, :]
                bc3 = bass.AP(
                    tensor=row.tensor, offset=row.offset,
                    ap=[list(row.ap[0])] + [[0, 64]] + [list(row.ap[1])],
                )
                nc.sync.dma_start(out=bct[0:64, :], in_=bc3)
                if h == 0:
                    nc.vector.tensor_mul(outh[0:64, :], pavs[0][0:64, :], bct[0:64, :])
                else:
                    oh1 = bcs_pool.tile([64, QBLK], mf, tag="oh1")
                    nc.vector.tensor_mul(oh1[:, :], pavs[1][0:64, :], bct[0:64, :])
                    nc.sync.dma_start(out=outh[64:128, :], in_=oh1[:, :])
            return outh

        def emit_tail(k, outh):
            bi, qb = divmod(k, nqb)
            g0 = bi * s + qb * QBLK
            # out-projection + bf16 partial writeback
            ysb = ysb_pool.tile([128, nnch, QBLK], bf16, tag="ysb")
            for n in range(nnch):
                py = pout_pool.tile([128, QBLK], f32, tag="pp")
                nc.tensor.matmul(
                    py[:, :], wo_sb[:, n, :], outh[:, :], start=True, stop=True
                )
                nc.vector.tensor_copy(ysb[:, n, :], py[:, :])
            nc.sync.dma_start(out=yTr[:, :, g0 : g0 + QBLK], in_=ysb[:, :, :])

        # Pipeline: proj(k) before attn(k-1), EXCEPT at sequence boundaries:
        # proj(bi+1, 0) overwrites kT / vones chunks that attn(bi, nqb-1) still
        # reads, so there attn(k-1) must be emitted first.
        qts = {}
        for k in range(nblk + 1):
            boundary = k % nqb == 0
            if boundary and k >= 1:
                emit_attn(k - 1, qts.pop(k - 1))
            if k < nblk:
                qts[k] = emit_proj(k)
            if not boundary and k >= 1:
                emit_attn(k - 1, qts.pop(k - 1))


# --------------------------------------------------------------------------
# host entry point
# --------------------------------------------------------------------------

def _shard_inputs(x, attention_mask, w_qkv, w_out, b, s, d):
    xT = np.ascontiguousarray(np.asarray(x, dtype=np.float32).reshape(b * s, d).T)
    w_qkv = np.asarray(w_qkv, dtype=np.float32)
    w_out = np.asarray(w_out, dtype=np.float32)
    cosT, sinTs = _rope_tables_T(s, HD, HPC)
    master = _master_mask(QBLK)
    am = np.asarray(attention_mask)
    # kmT[p, bi, t, 0] = mask value of key position t*128+p in sequence bi
    kmT = np.ascontiguousarray(
        (am != 0).astype(np.float32).reshape(b, s // JCH, JCH).transpose(2, 0, 1)[..., None]
    )
    perm = _perm64()
    cw = HPC * HD  # 128 columns per core
    in_maps = []
    for c in range(NCORES):
        sl = slice(c * cw, (c + 1) * cw)
        wq_c = w_qkv[:, 0 * d :][:, sl].copy()
        wk_c = w_qkv[:, 1 * d :][:, sl].copy()
        wv_c = w_qkv[:, 2 * d :][:, sl].copy()
        # permute q/k columns so rotate-half partners are adjacent partitions
        for h in range(HPC):
            blk = slice(h * HD, (h + 1) * HD)
            wq_c[:, blk] = wq_c[:, blk][:, perm]
            wk_c[:, blk] = wk_c[:, blk][:, perm]
        wqkv_c = np.ascontiguousarray(np.stack([wq_c, wk_c, wv_c], axis=1))  # [d,3,128]
        in_maps.append(
            {
                "xT": xT,
                "wqkv": wqkv_c,
                "wo": np.ascontiguousarray(w_out[sl, :]),
                "cosT": cosT,
                "sinTs": sinTs,
                "master": master,
                "kmT": kmT,
            }
        )
    return in_maps


_PROG_CACHE = {}


def _build_program(b, s, d, mm, has_padding=False):
    key = (b, s, d, mm, has_padding)
    if key in _PROG_CACHE:
        return _PROG_CACHE[key]
    import concourse.mybir as mybir
    from concourse import bacc
    from concourse.tile import TileContext

    f32 = mybir.dt.float32
    bf16 = mybir.dt.bfloat16
    mf = mybir.dt.float32r if mm == "f32r" else f32
    nc = bacc.Bacc("TRN2", target_bir_lowering=False, debug=False)
    bs = b * s
    ins = {
        "xT": nc.dram_tensor("xT", [d, bs], mf, kind="ExternalInput").ap(),
        "wqkv": nc.dram_tensor("wqkv", [d, 3, 128], mf, kind="ExternalInput").ap(),
        "wo": nc.dram_tensor("wo", [128, d], mf, kind="ExternalInput").ap(),
        "cosT": nc.dram_tensor("cosT", [128, s], f32, kind="ExternalInput").ap(),
        "sinTs": nc.dram_tensor("sinTs", [128, s], f32, kind="ExternalInput").ap(),
        "master": nc.dram_tensor("master", [128, 384 + QBLK], f32, kind="ExternalInput").ap(),
        "kmT": nc.dram_tensor("kmT", [128, b, s // JCH, 1], mf, kind="ExternalInput").ap(),
    }
    outs = {"yT": nc.dram_tensor("yT", [d, bs], bf16, kind="ExternalOutput").ap()}
    with TileContext(nc) as tc:
        emit(tc, outs, ins, b=b, s=s, d=d, mm=mm, has_padding=has_padding)
    nc.compile()
    _PROG_CACHE[key] = nc
    return nc


def kernel(x, attention_mask, w_qkv, w_out, *, mm="f32r", trace=False):
    from concourse import bass_utils

    b, s, d = x.shape
    has_padding = bool(np.any(np.asarray(attention_mask) == 0))
    nc = _build_program(b, s, d, mm, has_padding)
    in_maps = _shard_inputs(x, attention_mask, w_qkv, w_out, b, s, d)
    res = bass_utils.run_bass_kernel_spmd(
        nc, in_maps, core_ids=list(range(NCORES)), trace=trace
    )
    acc = res.results[0]["yT"].astype(np.float32)
    for c in range(1, NCORES):
        acc = acc + res.results[c]["yT"].astype(np.float32)
    out = np.ascontiguousarray(acc.T).reshape(b, s, d).astype(np.float32)
    if trace:
        return out, res
    return out
